# revision 2
# baseline (speedup 1.0000x reference)
"""VQ codebook kernel (euclidean cdist + argmax + gather + commitment loss +
straight-through) for 8 Trainium2 NeuronCores.

Strategy: data-parallel over tokens. z [8,2048,256] is sharded batch-wise
(core c gets batch c = 2048 tokens); the codebook [8192,256] is replicated.

Per core, the negative squared distance logits
    dist = 2*z@C.T - ||z||^2 - ||c||^2          [2048, 8192]
are computed on the TensorEngine in fp16 hi/lo 3-product form
(zh*ch + zh*cl + zl*ch, fp32 PSUM accumulation; max err ~2e-4 vs fp32)
plus one fp16 correction matmul carrying -||c||^2 (3-way fp16 split) and
-||z||^2 (3-way fp16 split) on 6 extra contraction rows.

argmax over K=8192 per token is hierarchical: reduce_max over 1024-wide
quarters -> [128, 8] chunk maxes -> max8/max_index pick the winning quarter
-> indirect-DMA gather of the winning quarter row from the dist output in
DRAM -> max_index within 1024 -> index = quarter*1024 + offset.

zq is gathered from the codebook by indirect DMA; zq_st = z + (zq - z) and
per-token sum((zq-z)^2) are computed on DVE/ACT. The loss mean is reduced
on the host (the cross-core "all-reduce").
"""
import os
import numpy as np
from contextlib import ExitStack

import concourse.bass as bass
import concourse.tile as tile
from concourse import bacc, mybir
from concourse.bass_utils import run_bass_kernel_spmd

F32 = mybir.dt.float32
F16 = mybir.dt.float16
U32 = mybir.dt.uint32
I32 = mybir.dt.int32
AF = mybir.ActivationFunctionType
ALU = mybir.AluOpType
AX = mybir.AxisListType

B, N, D, K = 8, 2048, 256, 8192
NCORES = 8
T = N  # tokens per core
TT = T // 128  # 16 token tiles
KC = K // 512  # 16 psum chunks
QW = 1024  # argmax quarter width
NQ = K // QW  # 8 quarters

_CACHE = {}


def _f16s(x):
    h = x.astype(np.float16)
    return h, (x - h.astype(np.float32)).astype(np.float32)


def _build():
    nc = bacc.Bacc("TRN2", target_bir_lowering=False, debug=False)
    dram = {}
    ins_spec = dict(
        ch0=(F16, (128, K)), ch1=(F16, (128, K)),
        cl0=(F16, (128, K)), cl1=(F16, (128, K)),
        corrr=(F16, (6, K)),
        zh0=(F16, (128, T)), zh1=(F16, (128, T)),
        zl0=(F16, (128, T)), zl1=(F16, (128, T)),
        corrl=(F16, (6, T)),
        zin=(F32, (T, D)),
        cb=(F32, (K, D)),
        tokb8=(U32, (128, 1)),
    )
    for name, (dt, shape) in ins_spec.items():
        dram[name] = nc.dram_tensor(name, list(shape), dt, kind="ExternalInput")
    outs_spec = dict(
        dist=(F32, (T, K)),
        zqst=(F32, (T, D)),
        idxs=(I32, (T,)),
        losss=(F32, (T,)),
    )
    for name, (dt, shape) in outs_spec.items():
        dram[name] = nc.dram_tensor(name, list(shape), dt, kind="ExternalOutput")

    dist_t = dram["dist"]
    # row view [T*NQ, QW] of dist for the quarter gather
    dist_rows = bass.AP(dist_t, 0, [[QW, T * NQ], [1, QW]])

    with tile.TileContext(nc) as tc, ExitStack() as ctx:
        cpool = ctx.enter_context(tc.tile_pool(name="cpool", bufs=1))
        stgp = ctx.enter_context(tc.tile_pool(name="stgp", bufs=3))
        psum = ctx.enter_context(tc.tile_pool(name="psum", bufs=8, space="PSUM"))
        work = ctx.enter_context(tc.tile_pool(name="work", bufs=2))

        sb = {}
        for name, (dt, shape) in ins_spec.items():
            if name in ("zin", "cb"):
                continue  # DRAM-resident (per-tile DMA / indirect gather)
            t_ = cpool.tile(list(shape), dt, tag=name, name=f"sb_{name}")
            nc.sync.dma_start(t_[:], dram[name].ap())
            sb[name] = t_

        idx_sb = cpool.tile([128, TT], I32, tag="idx_sb", name="idx_sb")
        loss_sb = cpool.tile([128, TT], F32, tag="loss_sb", name="loss_sb")

        prods = [("zh0", "ch0"), ("zh1", "ch1"), ("zh0", "cl0"),
                 ("zh1", "cl1"), ("zl0", "ch0"), ("zl1", "ch1")]

        for t in range(TT):
            ts_ = slice(t * 128, (t + 1) * 128)
            cm8 = work.tile([128, 8], F32, tag="cm8", name=f"cm8_{t}")
            for h in range(2):
                stg = stgp.tile([128, 4096], F32, tag="stg", name=f"stg_{t}_{h}")
                for kcl in range(8):
                    kc = h * 8 + kcl
                    ks = slice(kc * 512, (kc + 1) * 512)
                    pp = psum.tile([128, 512], F32, tag="pp", name=f"pp_{t}_{kc}")
                    for i, (zn, cn) in enumerate(prods):
                        nc.tensor.matmul(pp[:], sb[zn][:, ts_], sb[cn][:, ks],
                                         start=(i == 0), stop=False)
                    nc.tensor.matmul(pp[:], sb["corrl"][:, ts_],
                                     sb["corrr"][:, ks],
                                     start=False, stop=True)
                    nc.scalar.copy(stg[:, kcl * 512:(kcl + 1) * 512], pp[:])
                # per-quarter maxes for this half
                nc.vector.tensor_reduce(
                    out=cm8[:, h * 4:(h + 1) * 4],
                    in_=stg[:].rearrange("p (a b) -> p a b", a=4),
                    axis=AX.X, op=ALU.max)
                nc.sync.dma_start(
                    dist_t.ap()[ts_, h * 4096:(h + 1) * 4096], stg[:])
            # hierarchical argmax
            m8 = work.tile([128, 8], F32, tag="m8", name=f"m8_{t}")
            nc.vector.max(m8[:], cm8[:])
            c8 = work.tile([128, 8], U32, tag="c8", name=f"c8_{t}")
            nc.vector.max_index(c8[:], m8[:], cm8[:])
            r = work.tile([128, 1], U32, tag="r", name=f"r_{t}")
            nc.vector.tensor_scalar(out=r[:], in0=sb["tokb8"][:],
                                    scalar1=t * 128 * NQ, scalar2=None,
                                    op0=ALU.add)
            nc.vector.tensor_tensor(out=r[:], in0=r[:], in1=c8[:, 0:1],
                                    op=ALU.add)
            gath = work.tile([128, QW], F32, tag="gath", name=f"gath_{t}")
            nc.gpsimd.indirect_dma_start(
                gath[:], None, dist_rows,
                bass.IndirectOffsetOnAxis(ap=r[:], axis=0))
            w8 = work.tile([128, 8], U32, tag="w8", name=f"w8_{t}")
            nc.vector.max_index(w8[:], m8[:], gath[:])
            idxu = work.tile([128, 1], U32, tag="idxu", name=f"idxu_{t}")
            nc.vector.tensor_scalar(out=idxu[:], in0=c8[:, 0:1], scalar1=QW,
                                    scalar2=None, op0=ALU.mult)
            nc.vector.tensor_tensor(out=idxu[:], in0=idxu[:], in1=w8[:, 0:1],
                                    op=ALU.add)
            nc.vector.tensor_copy(idx_sb[:, t:t + 1], idxu[:])
            # zq gather + straight-through + loss
            zq = work.tile([128, D], F32, tag="zq", name=f"zq_{t}")
            nc.gpsimd.indirect_dma_start(
                zq[:], None, dram["cb"].ap(),
                bass.IndirectOffsetOnAxis(ap=idxu[:], axis=0))
            zt = work.tile([128, D], F32, tag="zt", name=f"zt_{t}")
            nc.sync.dma_start(zt[:], dram["zin"].ap()[ts_, :])
            tq = work.tile([128, D], F32, tag="tq", name=f"tq_{t}")
            nc.vector.tensor_tensor(out=tq[:], in0=zq[:], in1=zt[:],
                                    op=ALU.subtract)
            zqst_t = work.tile([128, D], F32, tag="zqst", name=f"zqst_{t}")
            nc.vector.tensor_tensor(out=zqst_t[:], in0=zt[:], in1=tq[:],
                                    op=ALU.add)
            nc.sync.dma_start(dram["zqst"].ap()[ts_, :], zqst_t[:])
            sqs = work.tile([128, D], F32, tag="sqs", name=f"sqs_{t}")
            nc.scalar.activation(sqs[:], tq[:], AF.Square, bias=0.0, scale=1.0,
                                 accum_out=loss_sb[:, t:t + 1])
        # collected outputs: flat[p*16 + t]
        nc.sync.dma_start(
            bass.AP(dram["idxs"], 0, [[TT, 128], [1, TT]]), idx_sb[:])
        nc.sync.dma_start(
            bass.AP(dram["losss"], 0, [[TT, 128], [1, TT]]), loss_sb[:])

    nc.compile()
    return nc


def _prep_shared(codebook):
    CT2 = (2.0 * codebook.T).astype(np.float32)  # [256, 8192]
    ch, rest = _f16s(CT2)
    cl, _ = _f16s(rest)
    c2 = np.sum(codebook.astype(np.float32) ** 2, axis=1, dtype=np.float32)
    c2a, r_ = _f16s(c2)
    c2b, r_ = _f16s(r_)
    c2c, _ = _f16s(r_)
    corrr = np.zeros((6, K), np.float16)
    corrr[0] = -c2a
    corrr[1] = -c2b
    corrr[2] = -c2c
    corrr[3:6] = 1.0
    tokb8 = (np.arange(128, dtype=np.uint32) * NQ)[:, None]
    return dict(
        ch0=ch[:128].copy(), ch1=ch[128:].copy(),
        cl0=cl[:128].copy(), cl1=cl[128:].copy(),
        corrr=corrr, cb=np.ascontiguousarray(codebook, np.float32),
        tokb8=np.ascontiguousarray(tokb8),
    )


def _prep_core(zc):
    zT = np.ascontiguousarray(zc.T, np.float32)  # [256, 2048]
    zh, rest = _f16s(zT)
    zl, _ = _f16s(rest)
    z2 = np.sum(zc.astype(np.float32) ** 2, axis=1, dtype=np.float32)
    z2a, r_ = _f16s(z2)
    z2b, r_ = _f16s(r_)
    z2c, _ = _f16s(r_)
    corrl = np.zeros((6, T), np.float16)
    corrl[0:3] = 1.0
    corrl[3] = -z2a
    corrl[4] = -z2b
    corrl[5] = -z2c
    return dict(
        zh0=zh[:128].copy(), zh1=zh[128:].copy(),
        zl0=zl[:128].copy(), zl1=zl[128:].copy(),
        corrl=corrl, zin=np.ascontiguousarray(zc, np.float32),
    )


def kernel(z, codebook):
    z = np.asarray(z, np.float32)
    codebook = np.asarray(codebook, np.float32)
    assert z.shape == (B, N, D) and codebook.shape == (K, D)

    if "nc" not in _CACHE:
        _CACHE["nc"] = _build()
    nc = _CACHE["nc"]

    shared = _prep_shared(codebook)
    in_maps = []
    for c in range(NCORES):
        m = dict(shared)
        m.update(_prep_core(z[c]))
        in_maps.append(m)

    trace = os.environ.get("VQ_TRACE", "0") == "1"
    res = run_bass_kernel_spmd(nc, in_maps, core_ids=list(range(NCORES)),
                               trace=trace)
    kernel.last_exec_time_ns = res.exec_time_ns
    kernel.last_results = res

    dist = np.stack([res.results[c]["dist"] for c in range(NCORES)])
    zqst = np.stack([res.results[c]["zqst"] for c in range(NCORES)])
    idxs = np.stack(
        [res.results[c]["idxs"].reshape(128, TT).T.reshape(T)
         for c in range(NCORES)])
    loss_total = np.float64(0.0)
    for c in range(NCORES):
        loss_total += np.float64(res.results[c]["losss"].sum(dtype=np.float64))
    vq_loss = np.float32(loss_total / (B * N * D))

    return (zqst.reshape(B, N, D), idxs.astype(np.int32).reshape(B, N),
            vq_loss, dist.reshape(B, N, K))
